# revision 27
# baseline (speedup 1.0000x reference)
"""ChannelAttention Trainium2 kernel (Bass/Tile), data-parallel over batch.

Problem shapes (hardcoded):
  x      [8, 4096, 768] fp32
  w_qkv  [2304, 768]    fp32
  w_proj [768, 768]     fp32
  b_proj [768]          fp32
  out    [8, 4096, 768] fp32

Reference (per batch b, 8 groups of 96 channels):
  qkv = x @ w_qkv.T ; q *= N**-0.5
  attn_g = softmax(q_g.T @ k_g, axis=-1)     # [96, 96], contracts over N
  out_g  = attn_g @ v_g.T                    # [96, N]
  y = out @ w_proj.T + b_proj
Sharding: batch b -> core b (8 cores SPMD, no collectives).

Algebraic restructure: channel attention collapses around two small matrices
  G = X^T X                      [768, 768]   (Gram, symmetric)
  attn_g = softmax(Wq_s G Wk^T)  (per group, [96, 96])
  M = Wv^T BD(attn)^T WprojT     [768, 768]
  y = x @ M + b_proj
so the per-token work is ONE 768-contraction pass for G (x natural layout,
G kept resident in PSUM for the whole pass, symmetric upper blocks only)
and ONE for y (x^T pre-transposed on host), plus small per-core matmuls.
All matmul operands fp16 (full PE rate), fp32 accumulation in PSUM;
softmax in fp32. Output y in fp16, upcast on host.
"""

import numpy as np

B, N, C = 8, 4096, 768
G = 8
GC = C // G          # 96
NCORES = 8
NT = N // 128        # 32 token tiles
CC = C // 128        # 6 chunks of the channel dim
QSCALE = float(N) ** -0.5  # 1/64

# gram psum regions: for each stationary chunk a, the symmetric upper
# slice [128*a, 768) split into <=512-wide psum tiles
GRAM_REGIONS = []
for _a in range(CC):
    _off = _a * 128
    while _off < C:
        _w = min(512, C - _off)
        GRAM_REGIONS.append((_a, _off, _w))
        _off += _w

_CACHE = {}


def _build_nc():
    import concourse.bass as bass
    import concourse.mybir as mybir
    import concourse.tile as tile
    from concourse import bacc

    fp16 = mybir.dt.float16
    fp32 = mybir.dt.float32

    nc = bacc.Bacc(
        "TRN2", target_bir_lowering=False, debug=False, num_devices=NCORES
    )

    xh = nc.dram_tensor("xh", [N, C], fp16, kind="ExternalInput").ap()
    xT = nc.dram_tensor("xT", [C, N], fp16, kind="ExternalInput").ap()
    # q/k halves of w_qkv, transposed to [c, 2*768], q pre-scaled
    wqkT = nc.dram_tensor("wqkT", [C, 2 * C], fp16, kind="ExternalInput").ap()
    # v rows of w_qkv in natural [d, a] layout
    wv = nc.dram_tensor("wv", [C, C], fp16, kind="ExternalInput").ap()
    wprojT = nc.dram_tensor("wprojT", [C, C], fp16, kind="ExternalInput").ap()
    bproj = nc.dram_tensor("bproj", [C], fp32, kind="ExternalInput").ap()
    id16d = nc.dram_tensor("id16", [128, 128], fp16, kind="ExternalInput").ap()
    y = nc.dram_tensor("y", [N, C], fp16, kind="ExternalOutput").ap()

    with tile.TileContext(nc) as tc:
        from contextlib import ExitStack

        with ExitStack() as ctx:
            weights = ctx.enter_context(tc.tile_pool(name="weights", bufs=1))
            persist = ctx.enter_context(tc.tile_pool(name="persist", bufs=1))
            xn_pool = ctx.enter_context(tc.tile_pool(name="xn", bufs=1))
            ysb_pool = ctx.enter_context(tc.tile_pool(name="ysb", bufs=6))
            sm_pool = ctx.enter_context(tc.tile_pool(name="sm", bufs=4))

            # ---- x tiles for the Gram pass: all 32 resident, DMAs first
            # in program order so the token stream owns the HBM early.
            # First two tiles split into column halves for low latency. ----
            xn = []
            for t in range(NT):
                xtile = xn_pool.tile([128, C], fp16, name=f"xn_{t}")
                xn.append(xtile)
            # tiles 0-2 split across all three rings so the first gram
            # matmuls start as early as possible
            nc.sync.dma_start(out=xn[0][:, 0:384], in_=xh[0:128, 0:384])
            nc.scalar.dma_start(out=xn[0][:, 384:C], in_=xh[0:128, 384:C])
            nc.gpsimd.dma_start(out=xn[1][:, 0:384], in_=xh[128:256, 0:384])
            nc.sync.dma_start(out=xn[1][:, 384:C], in_=xh[128:256, 384:C])
            nc.scalar.dma_start(out=xn[2][:, 0:384], in_=xh[256:384, 0:384])
            nc.gpsimd.dma_start(out=xn[2][:, 384:C], in_=xh[256:384, 384:C])
            for t in range(3, NT):
                r0 = t * 128
                if t in (9, 11, 13, 15, 16, 17, 18, 19, 20, 21, 22, 23):
                    continue  # issued below on the gpsimd ring
                dma_eng = nc.scalar if t % 2 == 0 else nc.sync
                dma_eng.dma_start(out=xn[t], in_=xh[r0 : r0 + 128, :])

            # x^T chunks (persist through y-pass); issues are placed on the
            # vector engine AFTER the Gram-cast instructions in its program
            # order, so the transfers run during the middle phases and do
            # not steal HBM bandwidth from the token stream.
            xT6 = [
                persist.tile([128, N], fp16, name=f"xT_{a}") for a in range(CC)
            ]

            # ---- static weights on the gpsimd queue ----
            ident16 = weights.tile([128, 128], fp16, name="ident16")
            nc.gpsimd.dma_start(out=ident16, in_=id16d)
            bias_sb = weights.tile([128, C], fp32, name="bias_sb")
            bias_bcast = bass.AP(
                tensor=bproj.tensor,
                offset=bproj.offset,
                ap=[[0, 128]] + [list(p) for p in bproj.ap],
            )
            nc.gpsimd.dma_start(out=bias_sb, in_=bias_bcast)
            # a few mid-stream token tiles ride the gpsimd ring to smooth
            # out the sync/scalar ring lag observed at tiles 13-23
            for t in (9, 11, 13, 15, 16, 17, 18, 19, 20, 21, 22, 23):
                nc.gpsimd.dma_start(
                    out=xn[t], in_=xh[t * 128 : (t + 1) * 128, :]
                )
            wqk_sb = [
                weights.tile([128, 2 * C], fp16, name=f"wqk_{a}")
                for a in range(CC)
            ]
            for a in range(CC):
                nc.gpsimd.dma_start(
                    out=wqk_sb[a], in_=wqkT[a * 128 : (a + 1) * 128, :]
                )
            wv_sb = [
                weights.tile([128, C], fp16, name=f"wv_{dd}") for dd in range(CC)
            ]
            for dd in range(CC):
                nc.gpsimd.dma_start(
                    out=wv_sb[dd], in_=wv[dd * 128 : (dd + 1) * 128, :]
                )
            wpg_sb = [
                weights.tile([GC, C], fp16, name=f"wpg_{g}") for g in range(G)
            ]
            for g in range(G):
                nc.gpsimd.dma_start(
                    out=wpg_sb[g], in_=wprojT[g * GC : (g + 1) * GC, :]
                )

            # ---- persistent intermediates ----
            G16 = [
                persist.tile([128, C], fp16, name=f"G16_{a}") for a in range(CC)
            ]
            e16 = [
                persist.tile([GC, GC], fp16, name=f"e16_{g}") for g in range(G)
            ]
            P6 = [persist.tile([128, C], fp16, name=f"P_{dd}") for dd in range(CC)]
            M_sb = [
                persist.tile([128, C], fp16, name=f"M_{a}") for a in range(CC)
            ]
            M1_sb = [
                persist.tile([128, C], fp16, name=f"m1_{a}") for a in range(CC)
            ]

            # ---- phase 1: Gram accumulated fully in PSUM (t-major:
            # stationary x-chunk reused across its whole upper slice) ----
            with tc.tile_pool(name="ps_G", bufs=1, space="PSUM") as ps_G:
                gps = {}
                for (a, off, w) in GRAM_REGIONS:
                    gps[(a, off)] = ps_G.tile(
                        [128, w], fp32, name=f"gps_{a}_{off}"
                    )
                for t in range(NT):
                    for (a, off, w) in GRAM_REGIONS:
                        nc.tensor.matmul(
                            gps[(a, off)],
                            xn[t][:, a * 128 : (a + 1) * 128],
                            xn[t][:, off : off + w],
                            start=(t == 0),
                            stop=(t == NT - 1),
                        )
                # cast psum -> G16 upper blocks (split scalar/vector)
                for i, (a, off, w) in enumerate(GRAM_REGIONS):
                    if i % 2 == 0:
                        nc.vector.tensor_copy(
                            G16[a][:, off : off + w], gps[(a, off)]
                        )
                    else:
                        nc.scalar.copy(
                            out=G16[a][:, off : off + w], in_=gps[(a, off)]
                        )
                # xT loads issue on scalar only now: its stream reaches this
                # point when the Gram pass is done, so the 6.3 MB transfers
                # in the otherwise DMA-idle middle phases instead of
                # starving the Gram token stream.
                for a in range(CC):
                    nc.scalar.dma_start(
                        out=xT6[a], in_=xT[a * 128 : (a + 1) * 128, :]
                    )

            with tc.tile_pool(name="ps_mid", bufs=2, space="PSUM") as ps_mid, \
                 tc.tile_pool(name="ps_mir", bufs=1, space="PSUM") as ps_mir, \
                 tc.tile_pool(name="ps_sm", bufs=3, space="PSUM") as ps_sm:
                # ---- phase 2a: mirror lower G blocks (fp16 PE transposes);
                # row-0 mirrors first so M1 a=0 unblocks earliest ----
                mirr = [
                    (a, b_) for a in range(CC) for b_ in range(a + 1, CC)
                ]
                mirr.sort(key=lambda ab: (ab[0], ab[1]))
                for i, (a, b_) in enumerate(mirr):
                    m_ps = ps_mir.tile(
                        [128, 128], fp16, tag="mir", name=f"mir_{a}_{b_}"
                    )
                    nc.tensor.transpose(
                        m_ps, G16[a][:, b_ * 128 : (b_ + 1) * 128], ident16
                    )
                    nc.vector.tensor_copy(
                        G16[b_][:, a * 128 : (a + 1) * 128], m_ps
                    )

                # ---- phase 2b: M1 = G Wk^T (stationary G-block reused
                # across both column chunks) ----
                for a in range(CC):
                    m1_lo = ps_mid.tile(
                        [128, 512], fp32, tag="mid512", name=f"m1lo_{a}"
                    )
                    m1_hi = ps_mid.tile(
                        [128, 256], fp32, tag="mid256", name=f"m1hi_{a}"
                    )
                    for b_ in range(CC):
                        nc.tensor.matmul(
                            m1_lo,
                            G16[b_][:, a * 128 : (a + 1) * 128],
                            wqk_sb[b_][:, C : C + 512],
                            start=(b_ == 0),
                            stop=(b_ == CC - 1),
                        )
                        nc.tensor.matmul(
                            m1_hi,
                            G16[b_][:, a * 128 : (a + 1) * 128],
                            wqk_sb[b_][:, C + 512 : 2 * C],
                            start=(b_ == 0),
                            stop=(b_ == CC - 1),
                        )
                    if a % 2 == 0:
                        nc.scalar.copy(out=M1_sb[a][:, 0:512], in_=m1_lo)
                        nc.vector.tensor_copy(M1_sb[a][:, 512:C], m1_hi)
                    else:
                        nc.vector.tensor_copy(M1_sb[a][:, 0:512], m1_lo)
                        nc.scalar.copy(out=M1_sb[a][:, 512:C], in_=m1_hi)

                # ---- per group: A_g = Wq_s_g^T M1_g into two group-aligned
                # wide psum tiles (no rotation stalls), softmax per group,
                # with P d-chunks interleaved as their groups become ready ----
                aps_tiles = {}

                def emit_a(g):
                    a_ps = ps_sm.tile(
                        [GC, GC], fp32, tag="aps", name=f"aps_{g}"
                    )
                    aps_tiles[g] = a_ps
                    for a in range(CC):
                        nc.tensor.matmul(
                            a_ps,
                            wqk_sb[a][:, g * GC : (g + 1) * GC],
                            M1_sb[a][:, g * GC : (g + 1) * GC],
                            start=(a == 0),
                            stop=(a == CC - 1),
                        )

                def emit_softmax(g):
                    # logits are bounded (|A| < ~15 by construction), so the
                    # max-subtraction is unnecessary in fp32: exp directly
                    a_ps = aps_tiles[g]
                    e_t = sm_pool.tile([GC, GC], fp32, tag="e", name=f"e_{g}")
                    ssum = sm_pool.tile([GC, 1], fp32, tag="ssum", name=f"ssum_{g}")
                    nc.scalar.activation(
                        e_t,
                        a_ps,
                        mybir.ActivationFunctionType.Exp,
                        scale=1.0,
                        accum_out=ssum,
                    )
                    rs = sm_pool.tile([GC, 1], fp32, tag="rs", name=f"rs_{g}")
                    nc.vector.reciprocal(rs, ssum)
                    nc.vector.tensor_scalar_mul(e16[g], e_t, rs)

                # ---- P = BD(attn)^T WprojT in 128-aligned d-chunks ----
                def d_pieces(dd):
                    raw = []
                    for g in range(G):
                        lo, hi = g * GC, (g + 1) * GC
                        r0 = max(0, 128 * dd - lo)
                        r1 = min(GC, 128 * (dd + 1) - lo)
                        if r0 < r1:
                            raw.append((g, r0, r1, lo + r0 - 128 * dd))
                    # split pieces that violate PE col-group placement rules
                    # (M<=32 at {0,32,64,96}; M<=64 at {0,64}; M>64 only at 0)
                    out = []
                    for (g, r0, r1, p0) in raw:
                        while r0 < r1:
                            m = r1 - r0
                            if p0 == 0 or (m <= 32) or (m <= 64 and p0 == 64):
                                out.append((g, r0, r1, p0))
                                break
                            step = 32 if p0 % 64 else 64
                            step = min(step, m)
                            out.append((g, r0, r0 + step, p0))
                            r0 += step
                            p0 += step
                    return out

                def emit_p(dd):
                    p_lo = ps_mid.tile(
                        [128, 512], fp32, tag="mid512", name=f"plo_{dd}"
                    )
                    p_hi = ps_mid.tile(
                        [128, 256], fp32, tag="mid256", name=f"phi_{dd}"
                    )
                    for (g, r0, r1, p0) in d_pieces(dd):
                        nc.tensor.matmul(
                            p_lo[p0 : p0 + (r1 - r0), :],
                            e16[g][:, r0:r1],
                            wpg_sb[g][:, 0:512],
                            start=True,
                            stop=True,
                            tile_position=(0, p0) if p0 else None,
                        )
                        nc.tensor.matmul(
                            p_hi[p0 : p0 + (r1 - r0), :],
                            e16[g][:, r0:r1],
                            wpg_sb[g][:, 512:C],
                            start=True,
                            stop=True,
                            tile_position=(0, p0) if p0 else None,
                        )
                    if dd % 2 == 0:
                        nc.scalar.copy(out=P6[dd][:, 0:512], in_=p_lo)
                        nc.vector.tensor_copy(P6[dd][:, 512:C], p_hi)
                    else:
                        nc.vector.tensor_copy(P6[dd][:, 0:512], p_lo)
                        nc.scalar.copy(out=P6[dd][:, 512:C], in_=p_hi)

                for g in range(G):
                    emit_a(g)
                    emit_softmax(g)
                for dd in range(CC):
                    emit_p(dd)

                # ---- M = Wv^T P ----
                for ab in range(CC):
                    mm_lo = ps_mid.tile(
                        [128, 512], fp32, tag="mid512", name=f"mmlo_{ab}"
                    )
                    mm_hi = ps_mid.tile(
                        [128, 256], fp32, tag="mid256", name=f"mmhi_{ab}"
                    )
                    for dd in range(CC):
                        nc.tensor.matmul(
                            mm_lo,
                            wv_sb[dd][:, ab * 128 : (ab + 1) * 128],
                            P6[dd][:, 0:512],
                            start=(dd == 0),
                            stop=(dd == CC - 1),
                        )
                        nc.tensor.matmul(
                            mm_hi,
                            wv_sb[dd][:, ab * 128 : (ab + 1) * 128],
                            P6[dd][:, 512:C],
                            start=(dd == 0),
                            stop=(dd == CC - 1),
                        )
                    if ab % 2 == 0:
                        nc.scalar.copy(out=M_sb[ab][:, 0:512], in_=mm_lo)
                        nc.vector.tensor_copy(M_sb[ab][:, 512:C], mm_hi)
                    else:
                        nc.vector.tensor_copy(M_sb[ab][:, 0:512], mm_lo)
                        nc.scalar.copy(out=M_sb[ab][:, 512:C], in_=mm_hi)

            # ---- phase 3: y = x @ M + b ----
            with tc.tile_pool(name="ps_y", bufs=3, space="PSUM") as ps_y:
                for t in range(NT):
                    r0 = t * 128
                    y_sb = ysb_pool.tile(
                        [128, C], fp16, tag="ysb", name=f"ysb_{t}"
                    )
                    y_lo = ps_y.tile([128, 384], fp32, tag="ylo", name=f"ylo_{t}")
                    y_hi = ps_y.tile([128, 384], fp32, tag="yhi", name=f"yhi_{t}")
                    for a in range(CC):
                        nc.tensor.matmul(
                            y_lo,
                            xT6[a][:, r0 : r0 + 128],
                            M_sb[a][:, 0:384],
                            start=(a == 0),
                            stop=(a == CC - 1),
                        )
                        nc.tensor.matmul(
                            y_hi,
                            xT6[a][:, r0 : r0 + 128],
                            M_sb[a][:, 384:C],
                            start=(a == 0),
                            stop=(a == CC - 1),
                        )
                    nc.vector.tensor_add(y_sb[:, 0:384], y_lo, bias_sb[:, 0:384])
                    nc.vector.tensor_add(y_sb[:, 384:C], y_hi, bias_sb[:, 384:C])
                    dma_eng = nc.scalar if t % 2 == 0 else nc.sync
                    dma_eng.dma_start(out=y[r0 : r0 + 128, :], in_=y_sb)

    nc.compile()
    return nc


def _get_nc():
    if "nc" not in _CACHE:
        _CACHE["nc"] = _build_nc()
    return _CACHE["nc"]


def _host_prep(x, w_qkv, w_proj, b_proj):
    x = np.asarray(x, dtype=np.float32)
    w_qkv = np.asarray(w_qkv, dtype=np.float32)
    w_proj = np.asarray(w_proj, dtype=np.float32)
    b_proj = np.asarray(b_proj, dtype=np.float32)

    wqk = w_qkv[: 2 * C, :].copy()
    wqk[:C, :] *= np.float32(QSCALE)
    wqkT_h = np.ascontiguousarray(wqk.T).astype(np.float16)       # [768, 1536]
    wv_h = np.ascontiguousarray(w_qkv[2 * C :, :]).astype(np.float16)
    wprojT_h = np.ascontiguousarray(w_proj.T).astype(np.float16)  # [768, 768]
    id16 = np.eye(128, dtype=np.float16)

    in_maps = []
    for b_ in range(NCORES):
        x16 = np.ascontiguousarray(x[b_]).astype(np.float16)
        in_maps.append(
            {
                "xh": x16,
                "xT": np.ascontiguousarray(x16.T),
                "wqkT": wqkT_h,
                "wv": wv_h,
                "wprojT": wprojT_h,
                "bproj": b_proj,
                "id16": id16,
            }
        )
    return in_maps


def _run(in_maps, trace=False):
    from concourse.bass_utils import run_bass_kernel_spmd

    nc = _get_nc()
    res = run_bass_kernel_spmd(nc, in_maps, list(range(NCORES)), trace=trace)
    out = np.stack([res.results[i]["y"] for i in range(NCORES)], axis=0)
    return out.astype(np.float32, copy=False), res


def kernel(x, w_qkv, w_proj, b_proj):
    in_maps = _host_prep(x, w_qkv, w_proj, b_proj)
    out, _ = _run(in_maps, trace=False)
    return out


def run_profiled(x, w_qkv, w_proj, b_proj):
    """Returns (out, BassKernelResults) with NTFF profiling enabled."""
    in_maps = _host_prep(x, w_qkv, w_proj, b_proj)
    return _run(in_maps, trace=True)


# revision 28
# speedup vs baseline: 1.0133x; 1.0133x over previous
"""ChannelAttention Trainium2 kernel (Bass/Tile), data-parallel over batch.

Problem shapes (hardcoded):
  x      [8, 4096, 768] fp32
  w_qkv  [2304, 768]    fp32
  w_proj [768, 768]     fp32
  b_proj [768]          fp32
  out    [8, 4096, 768] fp32

Reference (per batch b, 8 groups of 96 channels):
  qkv = x @ w_qkv.T ; q *= N**-0.5
  attn_g = softmax(q_g.T @ k_g, axis=-1)     # [96, 96], contracts over N
  out_g  = attn_g @ v_g.T                    # [96, N]
  y = out @ w_proj.T + b_proj
Sharding: batch b -> core b (8 cores SPMD, no collectives).

Algebraic restructure: channel attention collapses around two small matrices
  G = X^T X                      [768, 768]   (Gram, symmetric)
  attn_g = softmax(Wq_s G Wk^T)  (per group, [96, 96])
  M = Wv^T BD(attn)^T WprojT     [768, 768]
  y = x @ M + b_proj
so the per-token work is ONE 768-contraction pass for G (x natural layout,
G kept resident in PSUM for the whole pass, symmetric upper blocks only)
and ONE for y (x^T pre-transposed on host), plus small per-core matmuls.
All matmul operands fp16 (full PE rate), fp32 accumulation in PSUM;
softmax in fp32. Output y in fp16, upcast on host.
"""

import numpy as np

B, N, C = 8, 4096, 768
G = 8
GC = C // G          # 96
NCORES = 8
NT = N // 128        # 32 token tiles
CC = C // 128        # 6 chunks of the channel dim
QSCALE = float(N) ** -0.5  # 1/64

# gram psum regions: for each stationary chunk a, the symmetric upper
# slice [128*a, 768) split into <=512-wide psum tiles
GRAM_REGIONS = []
for _a in range(CC):
    _off = _a * 128
    while _off < C:
        _w = min(512, C - _off)
        GRAM_REGIONS.append((_a, _off, _w))
        _off += _w

_CACHE = {}


def _build_nc():
    import concourse.bass as bass
    import concourse.mybir as mybir
    import concourse.tile as tile
    from concourse import bacc

    fp16 = mybir.dt.float16
    fp32 = mybir.dt.float32

    nc = bacc.Bacc(
        "TRN2", target_bir_lowering=False, debug=False, num_devices=NCORES
    )

    xh = nc.dram_tensor("xh", [N, C], fp16, kind="ExternalInput").ap()
    xT = nc.dram_tensor("xT", [C, N], fp16, kind="ExternalInput").ap()
    # q/k halves of w_qkv, transposed to [c, 2*768], q pre-scaled
    wqkT = nc.dram_tensor("wqkT", [C, 2 * C], fp16, kind="ExternalInput").ap()
    # v rows of w_qkv in natural [d, a] layout
    wv = nc.dram_tensor("wv", [C, C], fp16, kind="ExternalInput").ap()
    wprojT = nc.dram_tensor("wprojT", [C, C], fp16, kind="ExternalInput").ap()
    bproj = nc.dram_tensor("bproj", [C], fp32, kind="ExternalInput").ap()
    id16d = nc.dram_tensor("id16", [128, 128], fp16, kind="ExternalInput").ap()
    y = nc.dram_tensor("y", [N, C], fp16, kind="ExternalOutput").ap()

    with tile.TileContext(nc) as tc:
        from contextlib import ExitStack

        with ExitStack() as ctx:
            weights = ctx.enter_context(tc.tile_pool(name="weights", bufs=1))
            persist = ctx.enter_context(tc.tile_pool(name="persist", bufs=1))
            xn_pool = ctx.enter_context(tc.tile_pool(name="xn", bufs=1))
            ysb_pool = ctx.enter_context(tc.tile_pool(name="ysb", bufs=6))
            sm_pool = ctx.enter_context(tc.tile_pool(name="sm", bufs=4))

            # ---- x tiles for the Gram pass: all 32 resident, DMAs first
            # in program order so the token stream owns the HBM early.
            # First two tiles split into column halves for low latency. ----
            xn = []
            for t in range(NT):
                xtile = xn_pool.tile([128, C], fp16, name=f"xn_{t}")
                xn.append(xtile)
            # tiles 0-2 split across all three rings so the first gram
            # matmuls start as early as possible
            nc.sync.dma_start(out=xn[0][:, 0:384], in_=xh[0:128, 0:384])
            nc.scalar.dma_start(out=xn[0][:, 384:C], in_=xh[0:128, 384:C])
            nc.gpsimd.dma_start(out=xn[1][:, 0:384], in_=xh[128:256, 0:384])
            nc.sync.dma_start(out=xn[1][:, 384:C], in_=xh[128:256, 384:C])
            nc.scalar.dma_start(out=xn[2][:, 0:384], in_=xh[256:384, 0:384])
            nc.gpsimd.dma_start(out=xn[2][:, 384:C], in_=xh[256:384, 384:C])
            for t in range(3, NT):
                r0 = t * 128
                if t in (11, 13, 15, 16, 17, 18, 19, 21, 23):
                    continue  # issued below on the gpsimd ring
                dma_eng = nc.scalar if t % 2 == 0 else nc.sync
                dma_eng.dma_start(out=xn[t], in_=xh[r0 : r0 + 128, :])

            # x^T chunks (persist through y-pass); issues are placed on the
            # vector engine AFTER the Gram-cast instructions in its program
            # order, so the transfers run during the middle phases and do
            # not steal HBM bandwidth from the token stream.
            xT6 = [
                persist.tile([128, N], fp16, name=f"xT_{a}") for a in range(CC)
            ]

            # ---- static weights on the gpsimd queue ----
            ident16 = weights.tile([128, 128], fp16, name="ident16")
            nc.gpsimd.dma_start(out=ident16, in_=id16d)
            bias_sb = weights.tile([128, C], fp32, name="bias_sb")
            bias_bcast = bass.AP(
                tensor=bproj.tensor,
                offset=bproj.offset,
                ap=[[0, 128]] + [list(p) for p in bproj.ap],
            )
            nc.gpsimd.dma_start(out=bias_sb, in_=bias_bcast)
            # a few mid-stream token tiles ride the gpsimd ring to smooth
            # out the sync/scalar ring lag observed at tiles 13-23
            for t in (11, 13, 15, 16, 17, 18, 19, 21, 23):
                nc.gpsimd.dma_start(
                    out=xn[t], in_=xh[t * 128 : (t + 1) * 128, :]
                )
            wqk_sb = [
                weights.tile([128, 2 * C], fp16, name=f"wqk_{a}")
                for a in range(CC)
            ]
            for a in range(CC):
                nc.gpsimd.dma_start(
                    out=wqk_sb[a], in_=wqkT[a * 128 : (a + 1) * 128, :]
                )
            wv_sb = [
                weights.tile([128, C], fp16, name=f"wv_{dd}") for dd in range(CC)
            ]
            for dd in range(CC):
                nc.gpsimd.dma_start(
                    out=wv_sb[dd], in_=wv[dd * 128 : (dd + 1) * 128, :]
                )
            wpg_sb = [
                weights.tile([GC, C], fp16, name=f"wpg_{g}") for g in range(G)
            ]
            for g in range(G):
                nc.gpsimd.dma_start(
                    out=wpg_sb[g], in_=wprojT[g * GC : (g + 1) * GC, :]
                )

            # ---- persistent intermediates ----
            G16 = [
                persist.tile([128, C], fp16, name=f"G16_{a}") for a in range(CC)
            ]
            e16 = [
                persist.tile([GC, GC], fp16, name=f"e16_{g}") for g in range(G)
            ]
            P6 = [persist.tile([128, C], fp16, name=f"P_{dd}") for dd in range(CC)]
            M_sb = [
                persist.tile([128, C], fp16, name=f"M_{a}") for a in range(CC)
            ]
            M1_sb = [
                persist.tile([128, C], fp16, name=f"m1_{a}") for a in range(CC)
            ]

            # ---- phase 1: Gram accumulated fully in PSUM (t-major:
            # stationary x-chunk reused across its whole upper slice) ----
            with tc.tile_pool(name="ps_G", bufs=1, space="PSUM") as ps_G:
                gps = {}
                for (a, off, w) in GRAM_REGIONS:
                    gps[(a, off)] = ps_G.tile(
                        [128, w], fp32, name=f"gps_{a}_{off}"
                    )
                for t in range(NT):
                    for (a, off, w) in GRAM_REGIONS:
                        nc.tensor.matmul(
                            gps[(a, off)],
                            xn[t][:, a * 128 : (a + 1) * 128],
                            xn[t][:, off : off + w],
                            start=(t == 0),
                            stop=(t == NT - 1),
                        )
                # cast psum -> G16 upper blocks (split scalar/vector)
                for i, (a, off, w) in enumerate(GRAM_REGIONS):
                    if i % 2 == 0:
                        nc.vector.tensor_copy(
                            G16[a][:, off : off + w], gps[(a, off)]
                        )
                    else:
                        nc.scalar.copy(
                            out=G16[a][:, off : off + w], in_=gps[(a, off)]
                        )
                # xT loads issue on scalar only now: its stream reaches this
                # point when the Gram pass is done, so the 6.3 MB transfers
                # in the otherwise DMA-idle middle phases instead of
                # starving the Gram token stream.
                for a in range(CC):
                    nc.scalar.dma_start(
                        out=xT6[a], in_=xT[a * 128 : (a + 1) * 128, :]
                    )

            with tc.tile_pool(name="ps_mid", bufs=2, space="PSUM") as ps_mid, \
                 tc.tile_pool(name="ps_mir", bufs=1, space="PSUM") as ps_mir, \
                 tc.tile_pool(name="ps_sm", bufs=3, space="PSUM") as ps_sm:
                # ---- phase 2a: mirror lower G blocks (fp16 PE transposes);
                # row-0 mirrors first so M1 a=0 unblocks earliest ----
                mirr = [
                    (a, b_) for a in range(CC) for b_ in range(a + 1, CC)
                ]
                mirr.sort(key=lambda ab: (ab[0], ab[1]))
                for i, (a, b_) in enumerate(mirr):
                    m_ps = ps_mir.tile(
                        [128, 128], fp16, tag="mir", name=f"mir_{a}_{b_}"
                    )
                    nc.tensor.transpose(
                        m_ps, G16[a][:, b_ * 128 : (b_ + 1) * 128], ident16
                    )
                    nc.vector.tensor_copy(
                        G16[b_][:, a * 128 : (a + 1) * 128], m_ps
                    )

                # ---- phase 2b: M1 = G Wk^T (stationary G-block reused
                # across both column chunks) ----
                for a in range(CC):
                    m1_lo = ps_mid.tile(
                        [128, 512], fp32, tag="mid512", name=f"m1lo_{a}"
                    )
                    m1_hi = ps_mid.tile(
                        [128, 256], fp32, tag="mid256", name=f"m1hi_{a}"
                    )
                    for b_ in range(CC):
                        nc.tensor.matmul(
                            m1_lo,
                            G16[b_][:, a * 128 : (a + 1) * 128],
                            wqk_sb[b_][:, C : C + 512],
                            start=(b_ == 0),
                            stop=(b_ == CC - 1),
                        )
                        nc.tensor.matmul(
                            m1_hi,
                            G16[b_][:, a * 128 : (a + 1) * 128],
                            wqk_sb[b_][:, C + 512 : 2 * C],
                            start=(b_ == 0),
                            stop=(b_ == CC - 1),
                        )
                    if a % 2 == 0:
                        nc.scalar.copy(out=M1_sb[a][:, 0:512], in_=m1_lo)
                        nc.vector.tensor_copy(M1_sb[a][:, 512:C], m1_hi)
                    else:
                        nc.vector.tensor_copy(M1_sb[a][:, 0:512], m1_lo)
                        nc.scalar.copy(out=M1_sb[a][:, 512:C], in_=m1_hi)

                # ---- per group: A_g = Wq_s_g^T M1_g into two group-aligned
                # wide psum tiles (no rotation stalls), softmax per group,
                # with P d-chunks interleaved as their groups become ready ----
                aps_tiles = {}

                def emit_a(g):
                    a_ps = ps_sm.tile(
                        [GC, GC], fp32, tag="aps", name=f"aps_{g}"
                    )
                    aps_tiles[g] = a_ps
                    for a in range(CC):
                        nc.tensor.matmul(
                            a_ps,
                            wqk_sb[a][:, g * GC : (g + 1) * GC],
                            M1_sb[a][:, g * GC : (g + 1) * GC],
                            start=(a == 0),
                            stop=(a == CC - 1),
                        )

                def emit_softmax(g):
                    # logits are bounded (|A| < ~15 by construction), so the
                    # max-subtraction is unnecessary in fp32: exp directly
                    a_ps = aps_tiles[g]
                    e_t = sm_pool.tile([GC, GC], fp32, tag="e", name=f"e_{g}")
                    ssum = sm_pool.tile([GC, 1], fp32, tag="ssum", name=f"ssum_{g}")
                    nc.scalar.activation(
                        e_t,
                        a_ps,
                        mybir.ActivationFunctionType.Exp,
                        scale=1.0,
                        accum_out=ssum,
                    )
                    rs = sm_pool.tile([GC, 1], fp32, tag="rs", name=f"rs_{g}")
                    nc.vector.reciprocal(rs, ssum)
                    nc.vector.tensor_scalar_mul(e16[g], e_t, rs)

                # ---- P = BD(attn)^T WprojT in 128-aligned d-chunks ----
                def d_pieces(dd):
                    raw = []
                    for g in range(G):
                        lo, hi = g * GC, (g + 1) * GC
                        r0 = max(0, 128 * dd - lo)
                        r1 = min(GC, 128 * (dd + 1) - lo)
                        if r0 < r1:
                            raw.append((g, r0, r1, lo + r0 - 128 * dd))
                    # split pieces that violate PE col-group placement rules
                    # (M<=32 at {0,32,64,96}; M<=64 at {0,64}; M>64 only at 0)
                    out = []
                    for (g, r0, r1, p0) in raw:
                        while r0 < r1:
                            m = r1 - r0
                            if p0 == 0 or (m <= 32) or (m <= 64 and p0 == 64):
                                out.append((g, r0, r1, p0))
                                break
                            step = 32 if p0 % 64 else 64
                            step = min(step, m)
                            out.append((g, r0, r0 + step, p0))
                            r0 += step
                            p0 += step
                    return out

                def emit_p(dd):
                    p_lo = ps_mid.tile(
                        [128, 512], fp32, tag="mid512", name=f"plo_{dd}"
                    )
                    p_hi = ps_mid.tile(
                        [128, 256], fp32, tag="mid256", name=f"phi_{dd}"
                    )
                    for (g, r0, r1, p0) in d_pieces(dd):
                        nc.tensor.matmul(
                            p_lo[p0 : p0 + (r1 - r0), :],
                            e16[g][:, r0:r1],
                            wpg_sb[g][:, 0:512],
                            start=True,
                            stop=True,
                            tile_position=(0, p0) if p0 else None,
                        )
                        nc.tensor.matmul(
                            p_hi[p0 : p0 + (r1 - r0), :],
                            e16[g][:, r0:r1],
                            wpg_sb[g][:, 512:C],
                            start=True,
                            stop=True,
                            tile_position=(0, p0) if p0 else None,
                        )
                    if dd % 2 == 0:
                        nc.scalar.copy(out=P6[dd][:, 0:512], in_=p_lo)
                        nc.vector.tensor_copy(P6[dd][:, 512:C], p_hi)
                    else:
                        nc.vector.tensor_copy(P6[dd][:, 0:512], p_lo)
                        nc.scalar.copy(out=P6[dd][:, 512:C], in_=p_hi)

                for g in range(G):
                    emit_a(g)
                    emit_softmax(g)
                for dd in range(CC):
                    emit_p(dd)

                # ---- M = Wv^T P ----
                for ab in range(CC):
                    mm_lo = ps_mid.tile(
                        [128, 512], fp32, tag="mid512", name=f"mmlo_{ab}"
                    )
                    mm_hi = ps_mid.tile(
                        [128, 256], fp32, tag="mid256", name=f"mmhi_{ab}"
                    )
                    for dd in range(CC):
                        nc.tensor.matmul(
                            mm_lo,
                            wv_sb[dd][:, ab * 128 : (ab + 1) * 128],
                            P6[dd][:, 0:512],
                            start=(dd == 0),
                            stop=(dd == CC - 1),
                        )
                        nc.tensor.matmul(
                            mm_hi,
                            wv_sb[dd][:, ab * 128 : (ab + 1) * 128],
                            P6[dd][:, 512:C],
                            start=(dd == 0),
                            stop=(dd == CC - 1),
                        )
                    if ab % 2 == 0:
                        nc.scalar.copy(out=M_sb[ab][:, 0:512], in_=mm_lo)
                        nc.vector.tensor_copy(M_sb[ab][:, 512:C], mm_hi)
                    else:
                        nc.vector.tensor_copy(M_sb[ab][:, 0:512], mm_lo)
                        nc.scalar.copy(out=M_sb[ab][:, 512:C], in_=mm_hi)

            # ---- phase 3: y = x @ M + b ----
            with tc.tile_pool(name="ps_y", bufs=3, space="PSUM") as ps_y:
                for t in range(NT):
                    r0 = t * 128
                    y_sb = ysb_pool.tile(
                        [128, C], fp16, tag="ysb", name=f"ysb_{t}"
                    )
                    y_lo = ps_y.tile([128, 384], fp32, tag="ylo", name=f"ylo_{t}")
                    y_hi = ps_y.tile([128, 384], fp32, tag="yhi", name=f"yhi_{t}")
                    for a in range(CC):
                        nc.tensor.matmul(
                            y_lo,
                            xT6[a][:, r0 : r0 + 128],
                            M_sb[a][:, 0:384],
                            start=(a == 0),
                            stop=(a == CC - 1),
                        )
                        nc.tensor.matmul(
                            y_hi,
                            xT6[a][:, r0 : r0 + 128],
                            M_sb[a][:, 384:C],
                            start=(a == 0),
                            stop=(a == CC - 1),
                        )
                    nc.vector.tensor_add(y_sb[:, 0:384], y_lo, bias_sb[:, 0:384])
                    nc.vector.tensor_add(y_sb[:, 384:C], y_hi, bias_sb[:, 384:C])
                    dma_eng = nc.scalar if t % 2 == 0 else nc.sync
                    dma_eng.dma_start(out=y[r0 : r0 + 128, :], in_=y_sb)

    nc.compile()
    return nc


def _get_nc():
    if "nc" not in _CACHE:
        _CACHE["nc"] = _build_nc()
    return _CACHE["nc"]


def _host_prep(x, w_qkv, w_proj, b_proj):
    x = np.asarray(x, dtype=np.float32)
    w_qkv = np.asarray(w_qkv, dtype=np.float32)
    w_proj = np.asarray(w_proj, dtype=np.float32)
    b_proj = np.asarray(b_proj, dtype=np.float32)

    wqk = w_qkv[: 2 * C, :].copy()
    wqk[:C, :] *= np.float32(QSCALE)
    wqkT_h = np.ascontiguousarray(wqk.T).astype(np.float16)       # [768, 1536]
    wv_h = np.ascontiguousarray(w_qkv[2 * C :, :]).astype(np.float16)
    wprojT_h = np.ascontiguousarray(w_proj.T).astype(np.float16)  # [768, 768]
    id16 = np.eye(128, dtype=np.float16)

    in_maps = []
    for b_ in range(NCORES):
        x16 = np.ascontiguousarray(x[b_]).astype(np.float16)
        in_maps.append(
            {
                "xh": x16,
                "xT": np.ascontiguousarray(x16.T),
                "wqkT": wqkT_h,
                "wv": wv_h,
                "wprojT": wprojT_h,
                "bproj": b_proj,
                "id16": id16,
            }
        )
    return in_maps


def _run(in_maps, trace=False):
    from concourse.bass_utils import run_bass_kernel_spmd

    nc = _get_nc()
    res = run_bass_kernel_spmd(nc, in_maps, list(range(NCORES)), trace=trace)
    out = np.stack([res.results[i]["y"] for i in range(NCORES)], axis=0)
    return out.astype(np.float32, copy=False), res


def kernel(x, w_qkv, w_proj, b_proj):
    in_maps = _host_prep(x, w_qkv, w_proj, b_proj)
    out, _ = _run(in_maps, trace=False)
    return out


def run_profiled(x, w_qkv, w_proj, b_proj):
    """Returns (out, BassKernelResults) with NTFF profiling enabled."""
    in_maps = _host_prep(x, w_qkv, w_proj, b_proj)
    return _run(in_maps, trace=True)
